# revision 66
# baseline (speedup 1.0000x reference)
"""Trainium2 Bass kernel for sparse CausalSelfAttention (8 full heads W=1024,
8 reduced-qk heads W=256), SPMD over 8 NeuronCores.

Sharding: core c -> batch c//4, head-group g=c%4 (full heads 2g,2g+1 and
reduced heads 2g,2g+1). fp16 activations/weights (fp32 PSUM accumulate).

v3: live-slice attention blocking (Q=512 full / Q=256 reduced) — score, exp
and PV instructions cover only the in-window column range of each key tile;
band edges handled by two shared 128x128 triangle masks applied with strided
two-region DVE ops. Score matmuls for the head pair run concurrently via PE
row tiling. The two reduced sub-blocks of each 512-T slab share one PSUM
accumulator and one normalize. Projection/cproj matmul chains are interleaved
into the attention phase boundaries (engine streams execute in emission
order, so PE work must be woven in manually where exp/normalize would stall).
"""

from itertools import chain

import numpy as np

import concourse.bacc as bacc
import concourse.mybir as mybir
from concourse import bass_utils
from concourse.ap import AP
from concourse.tile import TileContext

# problem constants (hardcoded; kernel.py must be self-contained)
B, T, C = 2, 2048, 1024
HDIM = 64           # full head dim (and v dim of reduced heads)
RDIM = 32           # reduced qk dim
WF, WR = 1024, 256  # windows
QF, QR = 512, 256   # query-block sizes
N_CORES = 8
NK = C // 128       # k-tiles over C contraction
PV_LAG = 2          # software-pipeline depth: PV matmuls lag exp

F32 = mybir.dt.float32
F16 = mybir.dt.float16
EXP = mybir.ActivationFunctionType.Exp
MASKS_ON_POOL = False  # apply band-edge masks on GpSimd instead of DVE


def host_masks():
    """[128, 2, 128] fp16: [:,0,c] upper triangle keep c<p, [:,1,c] lower
    keep c>=p (c = local query col within the 128-wide edge strip)."""
    p = np.arange(128)[:, None]
    c = np.arange(128)[None, :]
    m = np.zeros((128, 2, 128), np.float16)
    m[:, 0, :] = (c < p).astype(np.float16)
    m[:, 1, :] = (c >= p).astype(np.float16)
    return m


def _emit_body(nc, pools, aps, dbg=None):
    (wpool, xbpool, qkpool, pfpool, prpool, opool, rpool,
     ps_m, ps_s, ps_y) = pools
    xT, wqkv, wproj, masks, boot, out = aps

    # ---- boot tile (one startup DMA): wq weights k-major + x block-0
    # k-tiles 0:2, so the first projection chain starts earliest ----
    boot_sb = wpool.tile([128, 2048], F16, tag="boot")
    wq_sb = boot_sb[:, 0:1024].rearrange("p (k c) -> p k c", k=NK)
    # merged tile for the rest: cols 0:128 wk | 128:256 wqkr (krA krB qrA
    # qrB) | 256:512 wv
    wqkv_sb = wpool.tile([128, NK, 512], F16, tag="wqkv")
    wk_sb = wqkv_sb[:, :, 0:128]
    wqkr_sb = wqkv_sb[:, :, 128:256]
    wv_sb = wqkv_sb[:, :, 256:512]
    wproj_sb = wpool.tile([128, 2, C], F16, tag="wproj")
    m_sb = wpool.tile([128, 2, 128], F16, tag="masks")

    # persistent transposed activations [dim-stack, T]
    qTf = qkpool.tile([128, T], F16, tag="qTf")  # rows: hA q (64) | hB q (64)
    kTf = qkpool.tile([128, T], F16, tag="kTf")
    qTr = qkpool.tile([64, T], F16, tag="qTr")   # rows: qrA (32) | qrB (32)
    kTr = qkpool.tile([64, T], F16, tag="kTr")
    # v values + ones block: [128, T-tile, head, 128] (cols 64:128 = 1.0)
    v_sb = qkpool.tile([128, T // 128, 4, 128], F16, tag="v")
    nc.gpsimd.memset(v_sb[:, :, :, 64:128], 1.0)
    # attention outputs yT (normalized), stacked per pair
    yTf = qkpool.tile([128, T], F16, tag="yTf")
    yTr = qkpool.tile([128, T], F16, tag="yTr")

    xT3 = xT.rearrange("(k p) t -> p k t", p=128)
    wqkv3 = wqkv.rearrange("(k p) m -> p k m", p=128)

    def chain_qk(tb, w_sb, dsts):
        # one projection slab: psum = w.T @ x block, evacuated to dsts
        sl = slice(tb * 512, (tb + 1) * 512)
        xts = xts_all[tb]
        psum = ps_m.tile([128, 512], F32, tag="m")
        for k in range(NK):
            nc.tensor.matmul(psum[:], w_sb[:, k, :], xts[k],
                             start=(k == 0), stop=(k == NK - 1))
        for src_rows, dst in dsts:
            nc.vector.tensor_copy(dst[:, sl], psum[src_rows, :])

    def micro_chain_qk(tb, w_sb, dsts):
        # generator form of chain_qk: one matmul per next() — sized to the
        # ~200ns/kt PE starvation inside the ACT-bound reduced attention
        sl = slice(tb * 512, (tb + 1) * 512)
        xts = xts_all[tb]
        psum = ps_m.tile([128, 512], F32, tag="m")
        for k in range(NK):
            nc.tensor.matmul(psum[:], w_sb[:, k, :], xts[k],
                             start=(k == 0), stop=(k == NK - 1))
            yield
        for src_rows, dst in dsts:
            nc.vector.tensor_copy(dst[:, sl], psum[src_rows, :])

    def micro_cproj(tts):
        # generator form of cproj tiles: one matmul per next()
        for tt in tts:
            o_sb = opool.tile([128, 1, C], F16, tag="o")
            tsl = slice(tt * 128, (tt + 1) * 128)
            for nb in range(2):
                nsl = slice(nb * 512, (nb + 1) * 512)
                pso = ps_m.tile([128, 512], F32, tag="m")
                nc.tensor.matmul(pso[:], yTf[:, tsl], wproj_sb[:, 0, nsl],
                                 start=True, stop=False)
                yield
                nc.tensor.matmul(pso[:], yTr[:, tsl], wproj_sb[:, 1, nsl],
                                 start=False, stop=True)
                yield
                if nb == 0:
                    nc.scalar.copy(o_sb[:, 0, nsl], pso[:])
                else:
                    nc.vector.tensor_copy(o_sb[:, 0, nsl], pso[:])
            nc.sync.dma_start(out[tsl, :], o_sb[:, 0, :])

    def chain_v(tb, tt):
        gt = tb * 4 + tt  # global T-tile
        xts = xts_all[tb]
        psv = ps_m.tile([128, 256], F32, tag="m")
        for k in range(NK):
            nc.tensor.matmul(psv[:], xts[k][:, tt * 128:(tt + 1) * 128],
                             wv_sb[:, k, :],
                             start=(k == 0), stop=(k == NK - 1))
        nc.vector.tensor_copy(v_sb[:, gt, :, 0:64],
                              psv[:].rearrange("p (h d) -> p h d", h=4))

    def emit_mask_pair(pb, idx_a, col, mi):
        # one strided op over regions (idx_a, col:col+128) and
        # (idx_a+1, col+128:col+256), multiplied by triangle mask mi
        ap = [list(p) for p in pb.ap]
        pstride, idx_stride, h_stride = ap[0][0], ap[1][0], ap[2][0]
        cust = AP(pb.tensor, pb.offset + idx_a * idx_stride + col,
                  [[pstride, 128], [idx_stride + 128, 2], [h_stride, 2],
                   [1, 128]])
        mm = m_sb[:, mi, :].rearrange("p (a b q) -> p a b q", a=1, b=1)
        eng = nc.gpsimd if MASKS_ON_POOL else nc.vector
        eng.tensor_mul(cust, cust, mm.broadcast_to([128, 2, 2, 128]))

    def attn_sub(i0, Q, W, hw, heads, kT, qT, pb, py, off,
                 first_start, set_stop, fillers=None):
        """Scores+exp+mask+PV for one query sub-block into py[:, :, off:off+Q].
        first_start: this sub owns the PSUM has_written clear (its first PV
        uses start=True). set_stop: emit stop=True on the last PV."""
        kt_lo = max(0, i0 - W + 1) // 128
        kt_hi = (i0 + Q - 1) // 128
        kts = list(range(kt_lo, kt_hi + 1))
        n = len(kts)
        info = []
        mask_at = {}
        for idx, kt in enumerate(kts):
            d = i0 - 128 * kt
            lo = max(0, -d)
            hi = min(Q, W + 128 - d)
            info.append((idx, kt, lo, hi))
            u, l = W - d, -d
            if 0 <= u < Q and u % 256 == 0:
                mask_at.setdefault(idx + 1, []).append((idx, u, 0))
            if 0 <= l < Q and l % 256 == 0:
                mask_at.setdefault(idx + 1, []).append((idx, l, 1))
        # ready_at[idx]: loop position at which p[idx] is final (after its
        # exp, and after the pair mask op covering it, if any)
        ready_at = {idx: idx for idx, _, _, _ in info}
        for key, lst in mask_at.items():
            for idx_a, _, _ in lst:
                ready_at[idx_a] = key
                ready_at[idx_a + 1] = key
        # the first PV matmul must cover the full extent later PVs touch
        # (the has_written clear is per-bank): pick the first fully-live kt
        idx_ff = next(idx for idx, kt, lo, hi in info if lo == 0 and hi == Q)
        last_pv = next(i for i, _, _, _ in reversed(info) if i != idx_ff) \
            if n > 1 else idx_ff

        def emit_pv(idx, kt, lo, hi):
            for h in (0, 1):
                nc.tensor.matmul(
                    py[:, h, off + lo:off + hi], v_sb[:, kt, heads[h], :],
                    pb[:, idx, h, lo:hi],
                    start=(first_start and idx == idx_ff),
                    stop=(set_stop and idx == last_pv),
                    skip_group_check=True)

        pend = []
        pv_started = False
        for idx, kt, lo, hi in info:
            pss = ps_s.tile([128, 2, 512], F32, tag="s")
            ksl = slice(kt * 128, (kt + 1) * 128)
            qsl = slice(i0 + lo, i0 + hi)
            nc.tensor.matmul(pss[:, 0, lo:hi], kT[0:hw, ksl], qT[0:hw, qsl],
                             start=True, stop=True)
            nc.tensor.matmul(pss[:, 1, lo:hi], kT[hw:2 * hw, ksl],
                             qT[hw:2 * hw, qsl], start=True, stop=True)
            nc.scalar.activation(pb[:, idx, :, lo:hi], pss[:, :, lo:hi], EXP)
            for idx_a, col, mi in mask_at.get(idx, ()):
                emit_mask_pair(pb, idx_a, col, mi)
            if idx != idx_ff:
                pend.append((idx, kt, lo, hi))
            if not pv_started and idx >= ready_at[idx_ff]:
                emit_pv(*info[idx_ff])
                pv_started = True
            if pv_started:
                while len(pend) > PV_LAG:
                    emit_pv(*pend.pop(0))
            if fillers is not None:
                next(fillers, None)  # one micro-step of PE filler work
        if not pv_started:
            emit_pv(*info[idx_ff])
        for e in pend:
            emit_pv(*e)

    def normalize(py, yT, c0, width, rtag):
        r_sb = rpool.tile([64, 2, width], F32, tag=rtag)
        nc.vector.reciprocal(r_sb[:], py[64:128, :, 0:width])
        qsl = slice(c0, c0 + width)
        nc.vector.tensor_mul(yT[0:64, qsl], py[0:64, 0, 0:width],
                             r_sb[:, 0, :])
        nc.vector.tensor_mul(yT[64:128, qsl], py[0:64, 1, 0:width],
                             r_sb[:, 1, :])

    def full_attn(qb, micro=None):
        pb = pfpool.tile([128, 12, 2, QF], F16, tag="pf")
        py = ps_y.tile([128, 2, 512], F32, tag="y")
        attn_sub(QF * qb, QF, WF, 64, (0, 1), kTf, qTf, pb, py, 0,
                 True, True, fillers=micro)
        if micro is not None:
            for _ in micro:
                pass
        normalize(py, yTf, QF * qb, QF, "rf")

    def red_pair(tb, micro=None):
        py = ps_y.tile([128, 2, 512], F32, tag="y")
        for sub in (0, 1):
            pb = prpool.tile([128, 4, 2, QR], F16, tag="pr")
            attn_sub(QR * (2 * tb + sub), QR, WR, 32, (2, 3), kTr, qTr,
                     pb, py, 256 * sub, sub == 0, sub == 1, fillers=micro)
        if micro is not None:
            for _ in micro:
                pass
        normalize(py, yTr, 512 * tb, 512, "rr")

    def cproj_tile(tt, act_only=False, o2=None, j=0):
        # c_proj for one 128-row T-tile. act_only: keep DVE free for the
        # following attention phase's mask ops (they gate PV). o2: shared
        # two-tile output buffer — DMA fires once after the second tile.
        o_sb = o2 if o2 is not None else opool.tile([128, 1, C], F16,
                                                    tag="o")
        tsl = slice(tt * 128, (tt + 1) * 128)
        for nb in range(2):
            nsl = slice(nb * 512, (nb + 1) * 512)
            pso = ps_m.tile([128, 512], F32, tag="m")
            nc.tensor.matmul(pso[:], yTf[:, tsl], wproj_sb[:, 0, nsl],
                             start=True, stop=False)
            nc.tensor.matmul(pso[:], yTr[:, tsl], wproj_sb[:, 1, nsl],
                             start=False, stop=True)
            if nb == 0 or act_only:
                nc.scalar.copy(o_sb[:, j, nsl], pso[:])
            else:
                nc.vector.tensor_copy(o_sb[:, j, nsl], pso[:])
        if o2 is None:
            nc.sync.dma_start(out[tsl, :], o_sb[:, 0, :])

    def cproj_2tiles(tt0, act_only=False):
        o2 = opool.tile([128, 2, C], F16, tag="o2")
        for j in range(2):
            cproj_tile(tt0 + j, act_only=act_only, o2=o2, j=j)
        nc.sync.dma_start(
            out[tt0 * 128:(tt0 + 2) * 128, :].rearrange(
                "(j p) m -> p j m", p=128), o2[:])

    # ---- fused per-512-block loop, proj/cproj chains woven into the
    # attention phase boundaries ----
    xts_all = [None] * 4
    qk_dsts = {
        "wq": ((slice(0, 128), qTf),),
        "wk": ((slice(0, 128), kTf),),
        "wqkr": ((slice(0, 64), kTr), (slice(64, 128), qTr)),
    }
    for tb in range(T // 512):
        sl = slice(tb * 512, (tb + 1) * 512)
        if tb == 0:
            # one boot DMA (wq + x k0:2) unblocks the wq chain earliest
            nc.sync.dma_start(boot_sb[:], boot)
            xtb = xbpool.tile([128, NK, 512], F16, tag="xtb")
            nc.sync.dma_start(xtb[:, 2:NK, :], xT3[:, 2:NK, sl])
            nc.sync.dma_start(wqkv_sb[:], wqkv3)
            nc.sync.dma_start(m_sb[:],
                              masks.rearrange("p (a q) -> p a q", a=2))
            nc.sync.dma_start(wproj_sb[:],
                              wproj.rearrange("(k p) m -> p k m", p=128))
            xts_all[0] = [boot_sb[:, 1024:1536], boot_sb[:, 1536:2048]] + \
                [xtb[:, k, :] for k in range(2, NK)]
            chain_qk(0, wq_sb, qk_dsts["wq"])
            chain_qk(0, wk_sb, qk_dsts["wk"])
            chain_qk(0, wqkr_sb, qk_dsts["wqkr"])
            for tt in range(4):
                chain_v(0, tt)
        if tb + 1 < 4:  # prefetch next x block
            nsl = slice((tb + 1) * 512, (tb + 2) * 512)
            nxtb = xbpool.tile([128, NK, 512], F16, tag="xtb")
            nc.sync.dma_start(nxtb[:], xT3[:, :, nsl])
            xts_all[tb + 1] = [nxtb[:, k, :] for k in range(NK)]
        full_attn(tb, micro=(chain(
            micro_chain_qk(tb + 1, wq_sb, qk_dsts["wq"]),
            micro_chain_qk(tb + 1, wk_sb, qk_dsts["wk"]))
            if tb + 1 < 4 else None))
        if tb == 3:
            cproj_tile(4)  # 2 dep-free tiles fill the full-norm window;
            cproj_tile(5)  # more would queue DVE copies ahead of masks
        micro = micro_chain_qk(tb + 1, wqkr_sb, qk_dsts["wqkr"]) \
            if tb + 1 < 4 else None
        red_pair(tb, micro=micro)
        if tb == 1:  # dep-free cproj covers the red normalize window
            for tt in range(0, 4):
                cproj_tile(tt)
        elif tb == 3:
            for tt in range(6, 12):
                cproj_tile(tt)
        if tb + 1 < 4:
            for tt in range(4):
                chain_v(tb + 1, tt)
    for tt in range(12, 16):
        cproj_tile(tt)
    if dbg is not None:
        for name, tile in (("dqTf", qTf), ("dkTf", kTf), ("dqTr", qTr),
                           ("dkTr", kTr), ("dyTf", yTf), ("dyTr", yTr)):
            nc.sync.dma_start(dbg[name], tile[:])
        nc.sync.dma_start(dbg["dv"], v_sb[:].rearrange("p a h q -> p (a h q)"))


def _build_nc(reps=1, debug_outs=False):
    nc = bacc.Bacc(trn_type="TRN2", target_bir_lowering=False, debug=False,
                   num_devices=1)

    xT = nc.dram_tensor("xT", [C, T], F16, kind="ExternalInput").ap()
    wqkv = nc.dram_tensor("wqkv", [C, 512], F16, kind="ExternalInput").ap()
    wproj = nc.dram_tensor("wproj", [256, C], F16, kind="ExternalInput").ap()
    masks = nc.dram_tensor("masks", [128, 256], F16,
                           kind="ExternalInput").ap()
    boot = nc.dram_tensor("boot", [128, 2048], F16,
                          kind="ExternalInput").ap()
    out = nc.dram_tensor("o", [T, C], F16, kind="ExternalOutput").ap()
    aps = (xT, wqkv, wproj, masks, boot, out)
    dbg = None
    if debug_outs:
        dbg = {}
        for name, shape in (("dqTf", [128, T]), ("dkTf", [128, T]),
                            ("dqTr", [64, T]), ("dkTr", [64, T]),
                            ("dyTf", [128, T]), ("dyTr", [128, T]),
                            ("dv", [128, T * 4])):
            dbg[name] = nc.dram_tensor(name, shape, F16,
                                       kind="ExternalOutput").ap()

    with TileContext(nc) as tc:
        with (
            tc.tile_pool(name="wpool", bufs=1) as wpool,
            tc.tile_pool(name="xbpool", bufs=2) as xbpool,
            tc.tile_pool(name="qk", bufs=1) as qkpool,
            tc.tile_pool(name="pf", bufs=2) as pfpool,
            tc.tile_pool(name="pr", bufs=2) as prpool,
            tc.tile_pool(name="opool", bufs=4) as opool,
            tc.tile_pool(name="rpool", bufs=2) as rpool,
            tc.tile_pool(name="ps_m", bufs=2, space="PSUM") as ps_m,
            tc.tile_pool(name="ps_s", bufs=2, space="PSUM") as ps_s,
            tc.tile_pool(name="ps_y", bufs=1, space="PSUM") as ps_y,
        ):
            pools = (wpool, xbpool, qkpool, pfpool, prpool, opool, rpool,
                     ps_m, ps_s, ps_y)
            for _ in range(reps):
                _emit_body(nc, pools, aps, dbg=dbg)

    nc.compile()
    return nc


_NC_CACHE = {}


def _get_nc(reps=1):
    if reps not in _NC_CACHE:
        _NC_CACHE[reps] = _build_nc(reps)
    return _NC_CACHE[reps]


_MASKS = None


def make_in_maps(x, w_qkv_full, w_qk_red, w_v_red, w_proj):
    global _MASKS
    if _MASKS is None:
        _MASKS = np.ascontiguousarray(host_masks().reshape(128, 256))
    x = np.asarray(x, np.float32)
    w_qkv_full = np.asarray(w_qkv_full, np.float32)
    w_qk_red = np.asarray(w_qk_red, np.float32)
    w_v_red = np.asarray(w_v_red, np.float32)
    w_proj = np.asarray(w_proj, np.float32)
    sf = np.float32(1.0 / np.sqrt(HDIM))
    sr = np.float32(1.0 / np.sqrt(RDIM))
    in_maps = []
    for c in range(N_CORES):
        b, g = divmod(c, 4)
        hA, hB = 2 * g, 2 * g + 1
        wq = np.concatenate([w_qkv_full[:, 64 * hA:64 * hA + 64],
                             w_qkv_full[:, 64 * hB:64 * hB + 64]], 1) * sf
        wk = np.concatenate([w_qkv_full[:, 512 + 64 * hA:512 + 64 * hA + 64],
                             w_qkv_full[:, 512 + 64 * hB:512 + 64 * hB + 64]], 1)
        # packed reduced projection: rows 0:32 krA | 32:64 krB | qrA | qrB
        wqkr = np.concatenate(
            [w_qk_red[:, 256 + 32 * hA:256 + 32 * hA + 32],
             w_qk_red[:, 256 + 32 * hB:256 + 32 * hB + 32],
             w_qk_red[:, 32 * hA:32 * hA + 32] * sr,
             w_qk_red[:, 32 * hB:32 * hB + 32] * sr], 1)
        wv = np.concatenate([w_qkv_full[:, 1024 + 64 * hA:1024 + 64 * hA + 64],
                             w_qkv_full[:, 1024 + 64 * hB:1024 + 64 * hB + 64],
                             w_v_red[:, 64 * hA:64 * hA + 64],
                             w_v_red[:, 64 * hB:64 * hB + 64]], 1)
        wp = np.concatenate([w_proj[64 * hA:64 * hA + 64, :],
                             w_proj[64 * hB:64 * hB + 64, :],
                             w_proj[512 + 64 * hA:512 + 64 * hA + 64, :],
                             w_proj[512 + 64 * hB:512 + 64 * hB + 64, :]], 0)
        wqkv = np.concatenate([wk, wqkr, wv], 1)
        xT16 = np.ascontiguousarray(x[b].T).astype(np.float16)
        wq16 = wq.astype(np.float16)
        # boot: wq k-major [p, k, c] then x block-0 k-tiles 0:2 [p, j, t]
        boot = np.concatenate(
            [wq16.reshape(NK, 128, 128).transpose(1, 0, 2).reshape(128, 1024),
             xT16[0:256, 0:512].reshape(2, 128, 512)
             .transpose(1, 0, 2).reshape(128, 1024)], 1)
        in_maps.append({
            "xT": xT16,
            "wqkv": np.ascontiguousarray(wqkv).astype(np.float16),
            "wproj": np.ascontiguousarray(wp).astype(np.float16),
            "masks": _MASKS,
            "boot": np.ascontiguousarray(boot),
        })
    return in_maps


def kernel(x, w_qkv_full, w_qk_red, w_v_red, w_proj):
    nc = _get_nc()
    in_maps = make_in_maps(x, w_qkv_full, w_qk_red, w_v_red, w_proj)
    r = bass_utils.run_bass_kernel_spmd(nc, in_maps,
                                        core_ids=list(range(N_CORES)),
                                        trace=False)
    outs = [r.results[c]["o"] for c in range(N_CORES)]
    y = np.zeros((B, T, C), np.float32)
    for b in range(B):
        for j in range(4):
            y[b] += np.asarray(outs[4 * b + j], np.float32)
    return y


# revision 77
# speedup vs baseline: 1.2649x; 1.2649x over previous
"""Trainium2 Bass kernel for sparse CausalSelfAttention (8 full heads W=1024,
8 reduced-qk heads W=256), SPMD over 8 NeuronCores.

Sharding: core c -> batch c//4, head-group g=c%4 (full heads 2g,2g+1 and
reduced heads 2g,2g+1). fp16 activations/weights (fp32 PSUM accumulate).

v3: live-slice attention blocking (Q=512 full / Q=256 reduced) — score, exp
and PV instructions cover only the in-window column range of each key tile;
band edges handled by two shared 128x128 triangle masks applied with strided
two-region DVE ops. Score matmuls for the head pair run concurrently via PE
row tiling. The two reduced sub-blocks of each 512-T slab share one PSUM
accumulator and one normalize. Projection/cproj matmul chains are interleaved
into the attention phase boundaries (engine streams execute in emission
order, so PE work must be woven in manually where exp/normalize would stall).
"""

from itertools import chain

import numpy as np

import concourse.bacc as bacc
import concourse.mybir as mybir
from concourse import bass_utils
from concourse.ap import AP
from concourse.tile import TileContext

# problem constants (hardcoded; kernel.py must be self-contained)
B, T, C = 2, 2048, 1024
HDIM = 64           # full head dim (and v dim of reduced heads)
RDIM = 32           # reduced qk dim
WF, WR = 1024, 256  # windows
QF, QR = 512, 256   # query-block sizes
N_CORES = 8
NK = C // 128       # k-tiles over C contraction
PV_LAG = 2          # software-pipeline depth: PV matmuls lag exp

F32 = mybir.dt.float32
F16 = mybir.dt.float16
EXP = mybir.ActivationFunctionType.Exp
MASKS_ON_POOL = False  # apply band-edge masks on GpSimd instead of DVE


def host_masks():
    """[128, 2, 128] fp16: [:,0,c] upper triangle keep c<p, [:,1,c] lower
    keep c>=p (c = local query col within the 128-wide edge strip)."""
    p = np.arange(128)[:, None]
    c = np.arange(128)[None, :]
    m = np.zeros((128, 2, 128), np.float16)
    m[:, 0, :] = (c < p).astype(np.float16)
    m[:, 1, :] = (c >= p).astype(np.float16)
    return m


def _emit_body(nc, pools, aps, dbg=None):
    (wpool, xbpool, qkpool, pfpool, prpool, opool, rpool,
     ps_m, ps_s, ps_y) = pools
    xT, wqkv, wproj, masks, boot, out = aps

    # ---- boot tile: interleaved [wq_k0 | x_k0 | wq_k1 | x_k1 | wq_k2..7]
    # staged in three DMA chunks so the first matmul starts earliest ----
    boot_sb = wpool.tile([128, 2048], F16, tag="boot")
    wq_ks = ([boot_sb[:, 0:128], boot_sb[:, 640:768]] +
             [boot_sb[:, 1280 + 128 * j:1408 + 128 * j] for j in range(6)])
    boot_x = [boot_sb[:, 128:640], boot_sb[:, 768:1280]]
    # merged tile for the rest: cols 0:128 wk | 128:256 wqkr (krA krB qrA
    # qrB) | 256:512 wv
    wqkv_sb = wpool.tile([128, NK, 512], F16, tag="wqkv")
    wk_sb = wqkv_sb[:, :, 0:128]
    wqkr_sb = wqkv_sb[:, :, 128:256]
    wv_sb = wqkv_sb[:, :, 256:512]
    wproj_sb = wpool.tile([128, 2, C], F16, tag="wproj")
    m_sb = wpool.tile([128, 2, 128], F16, tag="masks")

    # persistent transposed activations [dim-stack, T]
    qTf = qkpool.tile([128, T], F16, tag="qTf")  # rows: hA q (64) | hB q (64)
    kTf = qkpool.tile([128, T], F16, tag="kTf")
    qTr = qkpool.tile([64, T], F16, tag="qTr")   # rows: qrA (32) | qrB (32)
    kTr = qkpool.tile([64, T], F16, tag="kTr")
    # v values + ones block: [128, T-tile, head, 128] (cols 64:128 = 1.0)
    v_sb = qkpool.tile([128, T // 128, 4, 128], F16, tag="v")
    nc.gpsimd.memset(v_sb[:, :, :, 64:128], 1.0)
    # attention outputs yT (normalized), stacked per pair
    yTf = qkpool.tile([128, T], F16, tag="yTf")
    yTr = qkpool.tile([128, T], F16, tag="yTr")

    xT3 = xT.rearrange("(k p) t -> p k t", p=128)
    wqkv3 = wqkv.rearrange("(k p) m -> p k m", p=128)

    def wsl(w_sb, k):
        return w_sb[k] if isinstance(w_sb, list) else w_sb[:, k, :]

    def chain_qk(tb, w_sb, dsts):
        # one projection slab: psum = w.T @ x block, evacuated to dsts
        sl = slice(tb * 512, (tb + 1) * 512)
        xts = xts_all[tb]
        psum = ps_m.tile([128, 512], F32, tag="m")
        for k in range(NK):
            nc.tensor.matmul(psum[:], wsl(w_sb, k), xts[k],
                             start=(k == 0), stop=(k == NK - 1))
        for src_rows, dst in dsts:
            nc.vector.tensor_copy(dst[:, sl], psum[src_rows, :])

    def micro_chain_qk(tb, w_sb, dsts):
        # generator form of chain_qk: one matmul per next() — sized to the
        # ~200ns/kt PE starvation inside the ACT-bound reduced attention
        sl = slice(tb * 512, (tb + 1) * 512)
        xts = xts_all[tb]
        psum = ps_m.tile([128, 512], F32, tag="m")
        for k in range(NK):
            nc.tensor.matmul(psum[:], wsl(w_sb, k), xts[k],
                             start=(k == 0), stop=(k == NK - 1))
            yield
        for src_rows, dst in dsts:
            nc.vector.tensor_copy(dst[:, sl], psum[src_rows, :])

    def micro_cproj(tts):
        # generator form of cproj tiles: one matmul per next()
        for tt in tts:
            o_sb = opool.tile([128, 1, C], F16, tag="o")
            tsl = slice(tt * 128, (tt + 1) * 128)
            for nb in range(2):
                nsl = slice(nb * 512, (nb + 1) * 512)
                pso = ps_m.tile([128, 512], F32, tag="m")
                nc.tensor.matmul(pso[:], yTf[:, tsl], wproj_sb[:, 0, nsl],
                                 start=True, stop=False)
                yield
                nc.tensor.matmul(pso[:], yTr[:, tsl], wproj_sb[:, 1, nsl],
                                 start=False, stop=True)
                yield
                if nb == 0:
                    nc.scalar.copy(o_sb[:, 0, nsl], pso[:])
                else:
                    nc.vector.tensor_copy(o_sb[:, 0, nsl], pso[:])
            nc.sync.dma_start(out[tsl, :], o_sb[:, 0, :])

    def chain_v(tb, tt):
        gt = tb * 4 + tt  # global T-tile
        xts = xts_all[tb]
        psv = ps_m.tile([128, 256], F32, tag="m")
        for k in range(NK):
            nc.tensor.matmul(psv[:], xts[k][:, tt * 128:(tt + 1) * 128],
                             wv_sb[:, k, :],
                             start=(k == 0), stop=(k == NK - 1))
        nc.vector.tensor_copy(v_sb[:, gt, :, 0:64],
                              psv[:].rearrange("p (h d) -> p h d", h=4))

    def emit_mask_pair(pb, idx_a, col, mi):
        # one strided op over regions (idx_a, col:col+128) and
        # (idx_a+1, col+128:col+256), multiplied by triangle mask mi
        ap = [list(p) for p in pb.ap]
        pstride, idx_stride, h_stride = ap[0][0], ap[1][0], ap[2][0]
        cust = AP(pb.tensor, pb.offset + idx_a * idx_stride + col,
                  [[pstride, 128], [idx_stride + 128, 2], [h_stride, 2],
                   [1, 128]])
        mm = m_sb[:, mi, :].rearrange("p (a b q) -> p a b q", a=1, b=1)
        eng = nc.gpsimd if MASKS_ON_POOL else nc.vector
        eng.tensor_mul(cust, cust, mm.broadcast_to([128, 2, 2, 128]))

    def attn_sub(i0, Q, W, hw, heads, kT, qT, pb, py, off,
                 first_start, set_stop, fillers=None):
        """Scores+exp+mask+PV for one query sub-block into py[:, :, off:off+Q].
        first_start: this sub owns the PSUM has_written clear (its first PV
        uses start=True). set_stop: emit stop=True on the last PV."""
        kt_lo = max(0, i0 - W + 1) // 128
        kt_hi = (i0 + Q - 1) // 128
        kts = list(range(kt_lo, kt_hi + 1))
        n = len(kts)
        info = []
        mask_at = {}
        for idx, kt in enumerate(kts):
            d = i0 - 128 * kt
            lo = max(0, -d)
            hi = min(Q, W + 128 - d)
            info.append((idx, kt, lo, hi))
            u, l = W - d, -d
            if 0 <= u < Q and u % 256 == 0:
                mask_at.setdefault(idx + 1, []).append((idx, u, 0))
            if 0 <= l < Q and l % 256 == 0:
                mask_at.setdefault(idx + 1, []).append((idx, l, 1))
        # ready_at[idx]: loop position at which p[idx] is final (after its
        # exp, and after the pair mask op covering it, if any)
        ready_at = {idx: idx for idx, _, _, _ in info}
        for key, lst in mask_at.items():
            for idx_a, _, _ in lst:
                ready_at[idx_a] = key
                ready_at[idx_a + 1] = key
        # the first PV matmul must cover the full extent later PVs touch
        # (the has_written clear is per-bank): pick the first fully-live kt
        idx_ff = next(idx for idx, kt, lo, hi in info if lo == 0 and hi == Q)
        last_pv = next(i for i, _, _, _ in reversed(info) if i != idx_ff) \
            if n > 1 else idx_ff

        def emit_pv(idx, kt, lo, hi):
            for h in (0, 1):
                nc.tensor.matmul(
                    py[:, h, off + lo:off + hi], v_sb[:, kt, heads[h], :],
                    pb[:, idx, h, lo:hi],
                    start=(first_start and idx == idx_ff),
                    stop=(set_stop and idx == last_pv),
                    skip_group_check=True)

        pend = []
        pv_started = False
        for idx, kt, lo, hi in info:
            pss = ps_s.tile([128, 2, 512], F32, tag="s")
            ksl = slice(kt * 128, (kt + 1) * 128)
            qsl = slice(i0 + lo, i0 + hi)
            nc.tensor.matmul(pss[:, 0, lo:hi], kT[0:hw, ksl], qT[0:hw, qsl],
                             start=True, stop=True)
            nc.tensor.matmul(pss[:, 1, lo:hi], kT[hw:2 * hw, ksl],
                             qT[hw:2 * hw, qsl], start=True, stop=True)
            nc.scalar.activation(pb[:, idx, :, lo:hi], pss[:, :, lo:hi], EXP)
            for idx_a, col, mi in mask_at.get(idx, ()):
                emit_mask_pair(pb, idx_a, col, mi)
            if idx != idx_ff:
                pend.append((idx, kt, lo, hi))
            if not pv_started and idx >= ready_at[idx_ff]:
                emit_pv(*info[idx_ff])
                pv_started = True
            if pv_started:
                while len(pend) > PV_LAG:
                    emit_pv(*pend.pop(0))
            if fillers is not None:
                next(fillers, None)  # one micro-step of PE filler work
        if not pv_started:
            emit_pv(*info[idx_ff])
        for e in pend:
            emit_pv(*e)

    def normalize(py, yT, c0, width, rtag):
        r_sb = rpool.tile([64, 2, width], F32, tag=rtag)
        nc.vector.reciprocal(r_sb[:], py[64:128, :, 0:width])
        qsl = slice(c0, c0 + width)
        nc.vector.tensor_mul(yT[0:64, qsl], py[0:64, 0, 0:width],
                             r_sb[:, 0, :])
        nc.vector.tensor_mul(yT[64:128, qsl], py[0:64, 1, 0:width],
                             r_sb[:, 1, :])

    def full_attn(qb, micro=None):
        pb = pfpool.tile([128, 12, 2, QF], F16, tag="pf")
        py = ps_y.tile([128, 2, 512], F32, tag="y")
        attn_sub(QF * qb, QF, WF, 64, (0, 1), kTf, qTf, pb, py, 0,
                 True, True, fillers=micro)
        if micro is not None:
            for _ in micro:
                pass
        normalize(py, yTf, QF * qb, QF, "rf")

    def red_pair(tb, micro=None):
        py = ps_y.tile([128, 2, 512], F32, tag="y")
        for sub in (0, 1):
            pb = prpool.tile([128, 4, 2, QR], F16, tag="pr")
            attn_sub(QR * (2 * tb + sub), QR, WR, 32, (2, 3), kTr, qTr,
                     pb, py, 256 * sub, sub == 0, sub == 1, fillers=micro)
        if micro is not None:
            for _ in micro:
                pass
        normalize(py, yTr, 512 * tb, 512, "rr")

    def cproj_tile(tt, act_only=False, o2=None, j=0):
        # c_proj for one 128-row T-tile. act_only: keep DVE free for the
        # following attention phase's mask ops (they gate PV). o2: shared
        # two-tile output buffer — DMA fires once after the second tile.
        o_sb = o2 if o2 is not None else opool.tile([128, 1, C], F16,
                                                    tag="o")
        tsl = slice(tt * 128, (tt + 1) * 128)
        for nb in range(2):
            nsl = slice(nb * 512, (nb + 1) * 512)
            pso = ps_m.tile([128, 512], F32, tag="m")
            nc.tensor.matmul(pso[:], yTf[:, tsl], wproj_sb[:, 0, nsl],
                             start=True, stop=False)
            nc.tensor.matmul(pso[:], yTr[:, tsl], wproj_sb[:, 1, nsl],
                             start=False, stop=True)
            if nb == 0 or act_only:
                nc.scalar.copy(o_sb[:, j, nsl], pso[:])
            else:
                nc.vector.tensor_copy(o_sb[:, j, nsl], pso[:])
        if o2 is None:
            nc.sync.dma_start(out[tsl, :], o_sb[:, 0, :])

    def cproj_2tiles(tt0, act_only=False):
        o2 = opool.tile([128, 2, C], F16, tag="o2")
        for j in range(2):
            cproj_tile(tt0 + j, act_only=act_only, o2=o2, j=j)
        nc.sync.dma_start(
            out[tt0 * 128:(tt0 + 2) * 128, :].rearrange(
                "(j p) m -> p j m", p=128), o2[:])

    # ---- fused per-512-block loop, proj/cproj chains woven into the
    # attention phase boundaries ----
    xts_all = [None] * 4
    qk_dsts = {
        "wq": ((slice(0, 128), qTf),),
        "wk": ((slice(0, 128), kTf),),
        "wqkr": ((slice(0, 64), kTr), (slice(64, 128), qTr)),
    }
    for tb in range(T // 512):
        sl = slice(tb * 512, (tb + 1) * 512)
        if tb == 0:
            # staged boot chunks: the first (wq_k0 + x_k0) unblocks the
            # very first matmul
            nc.sync.dma_start(boot_sb[:, 0:640], boot[:, 0:640])
            nc.sync.dma_start(boot_sb[:, 640:2048], boot[:, 640:2048])
            xtb = xbpool.tile([128, NK, 512], F16, tag="xtb")
            nc.sync.dma_start(xtb[:, 2:4, :], xT3[:, 2:4, sl])
            nc.sync.dma_start(xtb[:, 4:NK, :], xT3[:, 4:NK, sl])
            nc.sync.dma_start(wqkv_sb[:], wqkv3)
            nc.sync.dma_start(m_sb[:],
                              masks.rearrange("p (a q) -> p a q", a=2))
            nc.sync.dma_start(wproj_sb[:],
                              wproj.rearrange("(k p) m -> p k m", p=128))
            xts_all[0] = boot_x + [xtb[:, k, :] for k in range(2, NK)]
            chain_qk(0, wq_ks, qk_dsts["wq"])
            chain_qk(0, wk_sb, qk_dsts["wk"])
            chain_qk(0, wqkr_sb, qk_dsts["wqkr"])
            for tt in range(4):
                chain_v(0, tt)
        if tb + 1 < 4:  # prefetch next x block
            nsl = slice((tb + 1) * 512, (tb + 2) * 512)
            nxtb = xbpool.tile([128, NK, 512], F16, tag="xtb")
            nc.sync.dma_start(nxtb[:], xT3[:, :, nsl])
            xts_all[tb + 1] = [nxtb[:, k, :] for k in range(NK)]
        full_attn(tb, micro=(chain(
            micro_chain_qk(tb + 1, wq_ks, qk_dsts["wq"]),
            micro_chain_qk(tb + 1, wk_sb, qk_dsts["wk"]))
            if tb + 1 < 4 else None))
        if tb == 3:
            cproj_tile(4)  # 2 dep-free tiles fill the full-norm window;
            cproj_tile(5)  # more would queue DVE copies ahead of masks
        micro = micro_chain_qk(tb + 1, wqkr_sb, qk_dsts["wqkr"]) \
            if tb + 1 < 4 else None
        red_pair(tb, micro=micro)
        if tb == 1:  # dep-free cproj covers the red normalize window
            for tt in range(0, 4):
                cproj_tile(tt)
        elif tb == 3:
            for tt in range(6, 12):
                cproj_tile(tt)
        if tb + 1 < 4:
            for tt in range(4):
                chain_v(tb + 1, tt)
    for tt in range(12, 16):
        cproj_tile(tt)
    if dbg is not None:
        for name, tile in (("dqTf", qTf), ("dkTf", kTf), ("dqTr", qTr),
                           ("dkTr", kTr), ("dyTf", yTf), ("dyTr", yTr)):
            nc.sync.dma_start(dbg[name], tile[:])
        nc.sync.dma_start(dbg["dv"], v_sb[:].rearrange("p a h q -> p (a h q)"))


def _build_nc(reps=1, debug_outs=False):
    nc = bacc.Bacc(trn_type="TRN2", target_bir_lowering=False, debug=False,
                   num_devices=1)

    xT = nc.dram_tensor("xT", [C, T], F16, kind="ExternalInput").ap()
    wqkv = nc.dram_tensor("wqkv", [C, 512], F16, kind="ExternalInput").ap()
    wproj = nc.dram_tensor("wproj", [256, C], F16, kind="ExternalInput").ap()
    masks = nc.dram_tensor("masks", [128, 256], F16,
                           kind="ExternalInput").ap()
    boot = nc.dram_tensor("boot", [128, 2048], F16,
                          kind="ExternalInput").ap()
    out = nc.dram_tensor("o", [T, C], F16, kind="ExternalOutput").ap()
    aps = (xT, wqkv, wproj, masks, boot, out)
    dbg = None
    if debug_outs:
        dbg = {}
        for name, shape in (("dqTf", [128, T]), ("dkTf", [128, T]),
                            ("dqTr", [64, T]), ("dkTr", [64, T]),
                            ("dyTf", [128, T]), ("dyTr", [128, T]),
                            ("dv", [128, T * 4])):
            dbg[name] = nc.dram_tensor(name, shape, F16,
                                       kind="ExternalOutput").ap()

    with TileContext(nc) as tc:
        with (
            tc.tile_pool(name="wpool", bufs=1) as wpool,
            tc.tile_pool(name="xbpool", bufs=2) as xbpool,
            tc.tile_pool(name="qk", bufs=1) as qkpool,
            tc.tile_pool(name="pf", bufs=2) as pfpool,
            tc.tile_pool(name="pr", bufs=2) as prpool,
            tc.tile_pool(name="opool", bufs=4) as opool,
            tc.tile_pool(name="rpool", bufs=2) as rpool,
            tc.tile_pool(name="ps_m", bufs=2, space="PSUM") as ps_m,
            tc.tile_pool(name="ps_s", bufs=2, space="PSUM") as ps_s,
            tc.tile_pool(name="ps_y", bufs=1, space="PSUM") as ps_y,
        ):
            pools = (wpool, xbpool, qkpool, pfpool, prpool, opool, rpool,
                     ps_m, ps_s, ps_y)
            for _ in range(reps):
                _emit_body(nc, pools, aps, dbg=dbg)

    nc.compile()
    return nc


_NC_CACHE = {}


def _get_nc(reps=1):
    if reps not in _NC_CACHE:
        _NC_CACHE[reps] = _build_nc(reps)
    return _NC_CACHE[reps]


_MASKS = None


def make_in_maps(x, w_qkv_full, w_qk_red, w_v_red, w_proj):
    global _MASKS
    if _MASKS is None:
        _MASKS = np.ascontiguousarray(host_masks().reshape(128, 256))
    x = np.asarray(x, np.float32)
    w_qkv_full = np.asarray(w_qkv_full, np.float32)
    w_qk_red = np.asarray(w_qk_red, np.float32)
    w_v_red = np.asarray(w_v_red, np.float32)
    w_proj = np.asarray(w_proj, np.float32)
    sf = np.float32(1.0 / np.sqrt(HDIM))
    sr = np.float32(1.0 / np.sqrt(RDIM))
    in_maps = []
    for c in range(N_CORES):
        b, g = divmod(c, 4)
        hA, hB = 2 * g, 2 * g + 1
        wq = np.concatenate([w_qkv_full[:, 64 * hA:64 * hA + 64],
                             w_qkv_full[:, 64 * hB:64 * hB + 64]], 1) * sf
        wk = np.concatenate([w_qkv_full[:, 512 + 64 * hA:512 + 64 * hA + 64],
                             w_qkv_full[:, 512 + 64 * hB:512 + 64 * hB + 64]], 1)
        # packed reduced projection: rows 0:32 krA | 32:64 krB | qrA | qrB
        wqkr = np.concatenate(
            [w_qk_red[:, 256 + 32 * hA:256 + 32 * hA + 32],
             w_qk_red[:, 256 + 32 * hB:256 + 32 * hB + 32],
             w_qk_red[:, 32 * hA:32 * hA + 32] * sr,
             w_qk_red[:, 32 * hB:32 * hB + 32] * sr], 1)
        wv = np.concatenate([w_qkv_full[:, 1024 + 64 * hA:1024 + 64 * hA + 64],
                             w_qkv_full[:, 1024 + 64 * hB:1024 + 64 * hB + 64],
                             w_v_red[:, 64 * hA:64 * hA + 64],
                             w_v_red[:, 64 * hB:64 * hB + 64]], 1)
        wp = np.concatenate([w_proj[64 * hA:64 * hA + 64, :],
                             w_proj[64 * hB:64 * hB + 64, :],
                             w_proj[512 + 64 * hA:512 + 64 * hA + 64, :],
                             w_proj[512 + 64 * hB:512 + 64 * hB + 64, :]], 0)
        wqkv = np.concatenate([wk, wqkr, wv], 1)
        xT16 = np.ascontiguousarray(x[b].T).astype(np.float16)
        wq16 = wq.astype(np.float16)
        # boot, interleaved: [wq_k0 | x_k0 | wq_k1 | x_k1 | wq_k2..7]
        wqk = wq16.reshape(NK, 128, 128)
        boot = np.concatenate(
            [wqk[0], xT16[0:128, 0:512], wqk[1], xT16[128:256, 0:512],
             wqk[2:].transpose(1, 0, 2).reshape(128, 768)], 1)
        in_maps.append({
            "xT": xT16,
            "wqkv": np.ascontiguousarray(wqkv).astype(np.float16),
            "wproj": np.ascontiguousarray(wp).astype(np.float16),
            "masks": _MASKS,
            "boot": np.ascontiguousarray(boot),
        })
    return in_maps


def kernel(x, w_qkv_full, w_qk_red, w_v_red, w_proj):
    nc = _get_nc()
    in_maps = make_in_maps(x, w_qkv_full, w_qk_red, w_v_red, w_proj)
    r = bass_utils.run_bass_kernel_spmd(nc, in_maps,
                                        core_ids=list(range(N_CORES)),
                                        trace=False)
    outs = [r.results[c]["o"] for c in range(N_CORES)]
    y = np.zeros((B, T, C), np.float32)
    for b in range(B):
        for j in range(4):
            y[b] += np.asarray(outs[4 * b + j], np.float32)
    return y


# revision 81
# speedup vs baseline: 2.0480x; 1.6191x over previous
"""Trainium2 Bass kernel for sparse CausalSelfAttention (8 full heads W=1024,
8 reduced-qk heads W=256), SPMD over 8 NeuronCores.

Sharding: core c -> batch c//4, head-group g=c%4 (full heads 2g,2g+1 and
reduced heads 2g,2g+1). fp16 activations/weights (fp32 PSUM accumulate).

v3: live-slice attention blocking (Q=512 full / Q=256 reduced) — score, exp
and PV instructions cover only the in-window column range of each key tile;
band edges handled by two shared 128x128 triangle masks applied with strided
two-region DVE ops. Score matmuls for the head pair run concurrently via PE
row tiling. The two reduced sub-blocks of each 512-T slab share one PSUM
accumulator and one normalize. Projection/cproj matmul chains are interleaved
into the attention phase boundaries (engine streams execute in emission
order, so PE work must be woven in manually where exp/normalize would stall).
"""

from itertools import chain

import numpy as np

import concourse.bacc as bacc
import concourse.mybir as mybir
from concourse import bass_utils
from concourse.ap import AP
from concourse.tile import TileContext

# problem constants (hardcoded; kernel.py must be self-contained)
B, T, C = 2, 2048, 1024
HDIM = 64           # full head dim (and v dim of reduced heads)
RDIM = 32           # reduced qk dim
WF, WR = 1024, 256  # windows
QF, QR = 512, 256   # query-block sizes
N_CORES = 8
NK = C // 128       # k-tiles over C contraction
PV_LAG = 2          # software-pipeline depth: PV matmuls lag exp

F32 = mybir.dt.float32
F16 = mybir.dt.float16
EXP = mybir.ActivationFunctionType.Exp
MASKS_ON_POOL = False  # apply band-edge masks on GpSimd instead of DVE


def host_masks():
    """[128, 2, 128] fp16: [:,0,c] upper triangle keep c<p, [:,1,c] lower
    keep c>=p (c = local query col within the 128-wide edge strip)."""
    p = np.arange(128)[:, None]
    c = np.arange(128)[None, :]
    m = np.zeros((128, 2, 128), np.float16)
    m[:, 0, :] = (c < p).astype(np.float16)
    m[:, 1, :] = (c >= p).astype(np.float16)
    return m


def _emit_body(nc, pools, aps, dbg=None):
    (wpool, xbpool, qkpool, pfpool, prpool, opool, rpool,
     ps_m, ps_s, ps_y) = pools
    xT, wqkv, wproj, masks, boot, out = aps

    # ---- boot tile: interleaved [wq_k0 | x_k0 | wq_k1 | x_k1 | wq_k2..7]
    # staged in three DMA chunks so the first matmul starts earliest ----
    boot_sb = wpool.tile([128, 2048], F16, tag="boot")
    wq_ks = ([boot_sb[:, 0:128], boot_sb[:, 640:768]] +
             [boot_sb[:, 1280 + 128 * j:1408 + 128 * j] for j in range(6)])
    boot_x = [boot_sb[:, 128:640], boot_sb[:, 768:1280]]
    # merged tile for the rest: cols 0:128 wk | 128:256 wqkr (krA krB qrA
    # qrB) | 256:512 wv
    wqkv_sb = wpool.tile([128, NK, 512], F16, tag="wqkv")
    wk_sb = wqkv_sb[:, :, 0:128]
    wqkr_sb = wqkv_sb[:, :, 128:256]
    wv_sb = wqkv_sb[:, :, 256:512]
    wproj_sb = wpool.tile([128, 2, C], F16, tag="wproj")
    m_sb = wpool.tile([128, 2, 128], F16, tag="masks")

    # persistent transposed activations [dim-stack, T]
    qTf = qkpool.tile([128, T], F16, tag="qTf")  # rows: hA q (64) | hB q (64)
    kTf = qkpool.tile([128, T], F16, tag="kTf")
    qTr = qkpool.tile([64, T], F16, tag="qTr")   # rows: qrA (32) | qrB (32)
    kTr = qkpool.tile([64, T], F16, tag="kTr")
    # v values + ones block: [128, T-tile, head, 128] (cols 64:128 = 1.0)
    v_sb = qkpool.tile([128, T // 128, 4, 128], F16, tag="v")
    nc.gpsimd.memset(v_sb[:, :, :, 64:128], 1.0)
    # attention outputs yT (normalized), stacked per pair
    yTf = qkpool.tile([128, T], F16, tag="yTf")
    yTr = qkpool.tile([128, T], F16, tag="yTr")

    xT3 = xT.rearrange("(k p) t -> p k t", p=128)
    wqkv3 = wqkv.rearrange("(k p) m -> p k m", p=128)

    def wsl(w_sb, k):
        return w_sb[k] if isinstance(w_sb, list) else w_sb[:, k, :]

    def chain_qk(tb, w_sb, dsts):
        # one projection slab: psum = w.T @ x block, evacuated to dsts
        sl = slice(tb * 512, (tb + 1) * 512)
        xts = xts_all[tb]
        psum = ps_m.tile([128, 512], F32, tag="m")
        for k in range(NK):
            nc.tensor.matmul(psum[:], wsl(w_sb, k), xts[k],
                             start=(k == 0), stop=(k == NK - 1))
        for src_rows, dst in dsts:
            nc.vector.tensor_copy(dst[:, sl], psum[src_rows, :])

    def micro_chain_qk(tb, w_sb, dsts):
        # generator form of chain_qk: one matmul per next() — sized to the
        # ~200ns/kt PE starvation inside the ACT-bound reduced attention
        sl = slice(tb * 512, (tb + 1) * 512)
        xts = xts_all[tb]
        psum = ps_m.tile([128, 512], F32, tag="m")
        for k in range(NK):
            nc.tensor.matmul(psum[:], wsl(w_sb, k), xts[k],
                             start=(k == 0), stop=(k == NK - 1))
            yield
        for src_rows, dst in dsts:
            nc.vector.tensor_copy(dst[:, sl], psum[src_rows, :])

    def micro_cproj(tts):
        # generator form of cproj tiles: one matmul per next()
        for tt in tts:
            o_sb = opool.tile([128, 1, C], F16, tag="o")
            tsl = slice(tt * 128, (tt + 1) * 128)
            for nb in range(2):
                nsl = slice(nb * 512, (nb + 1) * 512)
                pso = ps_m.tile([128, 512], F32, tag="m")
                nc.tensor.matmul(pso[:], yTf[:, tsl], wproj_sb[:, 0, nsl],
                                 start=True, stop=False)
                yield
                nc.tensor.matmul(pso[:], yTr[:, tsl], wproj_sb[:, 1, nsl],
                                 start=False, stop=True)
                yield
                if nb == 0:
                    nc.scalar.copy(o_sb[:, 0, nsl], pso[:])
                else:
                    nc.vector.tensor_copy(o_sb[:, 0, nsl], pso[:])
            nc.sync.dma_start(out[tsl, :], o_sb[:, 0, :])

    def chain_v(tb, tt):
        gt = tb * 4 + tt  # global T-tile
        xts = xts_all[tb]
        psv = ps_m.tile([128, 256], F32, tag="m")
        for k in range(NK):
            nc.tensor.matmul(psv[:], xts[k][:, tt * 128:(tt + 1) * 128],
                             wv_sb[:, k, :],
                             start=(k == 0), stop=(k == NK - 1))
        nc.vector.tensor_copy(v_sb[:, gt, :, 0:64],
                              psv[:].rearrange("p (h d) -> p h d", h=4))

    def emit_mask_pair(pb, idx_a, col, mi):
        # one strided op over regions (idx_a, col:col+128) and
        # (idx_a+1, col+128:col+256), multiplied by triangle mask mi
        ap = [list(p) for p in pb.ap]
        pstride, idx_stride, h_stride = ap[0][0], ap[1][0], ap[2][0]
        cust = AP(pb.tensor, pb.offset + idx_a * idx_stride + col,
                  [[pstride, 128], [idx_stride + 128, 2], [h_stride, 2],
                   [1, 128]])
        mm = m_sb[:, mi, :].rearrange("p (a b q) -> p a b q", a=1, b=1)
        eng = nc.gpsimd if MASKS_ON_POOL else nc.vector
        eng.tensor_mul(cust, cust, mm.broadcast_to([128, 2, 2, 128]))

    def attn_sub(i0, Q, W, hw, heads, kT, qT, pb, py, off,
                 first_start, set_stop, fillers=None):
        """Scores+exp+mask+PV for one query sub-block into py[:, :, off:off+Q].
        first_start: this sub owns the PSUM has_written clear (its first PV
        uses start=True). set_stop: emit stop=True on the last PV."""
        kt_lo = max(0, i0 - W + 1) // 128
        kt_hi = (i0 + Q - 1) // 128
        kts = list(range(kt_lo, kt_hi + 1))
        n = len(kts)
        info = []
        mask_at = {}
        for idx, kt in enumerate(kts):
            d = i0 - 128 * kt
            lo = max(0, -d)
            hi = min(Q, W + 128 - d)
            info.append((idx, kt, lo, hi))
            u, l = W - d, -d
            if 0 <= u < Q and u % 256 == 0:
                mask_at.setdefault(idx + 1, []).append((idx, u, 0))
            if 0 <= l < Q and l % 256 == 0:
                mask_at.setdefault(idx + 1, []).append((idx, l, 1))
        # ready_at[idx]: loop position at which p[idx] is final (after its
        # exp, and after the pair mask op covering it, if any)
        ready_at = {idx: idx for idx, _, _, _ in info}
        for key, lst in mask_at.items():
            for idx_a, _, _ in lst:
                ready_at[idx_a] = key
                ready_at[idx_a + 1] = key
        # the first PV matmul must cover the full extent later PVs touch
        # (the has_written clear is per-bank): pick the first fully-live kt
        idx_ff = next(idx for idx, kt, lo, hi in info if lo == 0 and hi == Q)
        last_pv = next(i for i, _, _, _ in reversed(info) if i != idx_ff) \
            if n > 1 else idx_ff

        def emit_pv(idx, kt, lo, hi):
            for h in (0, 1):
                nc.tensor.matmul(
                    py[:, h, off + lo:off + hi], v_sb[:, kt, heads[h], :],
                    pb[:, idx, h, lo:hi],
                    start=(first_start and idx == idx_ff),
                    stop=(set_stop and idx == last_pv),
                    skip_group_check=True)

        pend = []
        pv_started = False
        for idx, kt, lo, hi in info:
            pss = ps_s.tile([128, 2, 512], F32, tag="s")
            ksl = slice(kt * 128, (kt + 1) * 128)
            qsl = slice(i0 + lo, i0 + hi)
            nc.tensor.matmul(pss[:, 0, lo:hi], kT[0:hw, ksl], qT[0:hw, qsl],
                             start=True, stop=True)
            nc.tensor.matmul(pss[:, 1, lo:hi], kT[hw:2 * hw, ksl],
                             qT[hw:2 * hw, qsl], start=True, stop=True)
            nc.scalar.activation(pb[:, idx, :, lo:hi], pss[:, :, lo:hi], EXP)
            for idx_a, col, mi in mask_at.get(idx, ()):
                emit_mask_pair(pb, idx_a, col, mi)
            if idx != idx_ff:
                pend.append((idx, kt, lo, hi))
            if not pv_started and idx >= ready_at[idx_ff]:
                emit_pv(*info[idx_ff])
                pv_started = True
            if pv_started:
                while len(pend) > PV_LAG:
                    emit_pv(*pend.pop(0))
            if fillers is not None:
                next(fillers, None)  # one micro-step of PE filler work
        if not pv_started:
            emit_pv(*info[idx_ff])
        for e in pend:
            emit_pv(*e)

    def normalize(py, yT, c0, width, rtag):
        r_sb = rpool.tile([64, 2, width], F32, tag=rtag)
        nc.vector.reciprocal(r_sb[:], py[64:128, :, 0:width])
        qsl = slice(c0, c0 + width)
        nc.vector.tensor_mul(yT[0:64, qsl], py[0:64, 0, 0:width],
                             r_sb[:, 0, :])
        nc.vector.tensor_mul(yT[64:128, qsl], py[0:64, 1, 0:width],
                             r_sb[:, 1, :])

    def full_attn(qb, micro=None):
        pb = pfpool.tile([128, 12, 2, QF], F16, tag="pf")
        py = ps_y.tile([128, 2, 512], F32, tag="y")
        attn_sub(QF * qb, QF, WF, 64, (0, 1), kTf, qTf, pb, py, 0,
                 True, True, fillers=micro)
        if micro is not None:
            for _ in micro:
                pass
        normalize(py, yTf, QF * qb, QF, "rf")

    def red_pair(tb, micro=None):
        py = ps_y.tile([128, 2, 512], F32, tag="y")
        for sub in (0, 1):
            pb = prpool.tile([128, 4, 2, QR], F16, tag="pr")
            attn_sub(QR * (2 * tb + sub), QR, WR, 32, (2, 3), kTr, qTr,
                     pb, py, 256 * sub, sub == 0, sub == 1, fillers=micro)
        if micro is not None:
            for _ in micro:
                pass
        normalize(py, yTr, 512 * tb, 512, "rr")

    def cproj_tile(tt, act_only=False, o2=None, j=0, dma_eng=None):
        # c_proj for one 128-row T-tile. act_only: keep DVE free for the
        # following attention phase's mask ops (they gate PV). o2: shared
        # two-tile output buffer — DMA fires once after the second tile.
        o_sb = o2 if o2 is not None else opool.tile([128, 1, C], F16,
                                                    tag="o")
        tsl = slice(tt * 128, (tt + 1) * 128)
        for nb in range(2):
            nsl = slice(nb * 512, (nb + 1) * 512)
            pso = ps_m.tile([128, 512], F32, tag="m")
            nc.tensor.matmul(pso[:], yTf[:, tsl], wproj_sb[:, 0, nsl],
                             start=True, stop=False)
            nc.tensor.matmul(pso[:], yTr[:, tsl], wproj_sb[:, 1, nsl],
                             start=False, stop=True)
            if nb == 0 or act_only:
                nc.scalar.copy(o_sb[:, j, nsl], pso[:])
            else:
                nc.vector.tensor_copy(o_sb[:, j, nsl], pso[:])
        if o2 is None:
            (dma_eng or nc.sync).dma_start(out[tsl, :], o_sb[:, 0, :])

    def cproj_2tiles(tt0, act_only=False):
        o2 = opool.tile([128, 2, C], F16, tag="o2")
        for j in range(2):
            cproj_tile(tt0 + j, act_only=act_only, o2=o2, j=j)
        nc.sync.dma_start(
            out[tt0 * 128:(tt0 + 2) * 128, :].rearrange(
                "(j p) m -> p j m", p=128), o2[:])

    # ---- fused per-512-block loop, proj/cproj chains woven into the
    # attention phase boundaries ----
    xts_all = [None] * 4
    qk_dsts = {
        "wq": ((slice(0, 128), qTf),),
        "wk": ((slice(0, 128), kTf),),
        "wqkr": ((slice(0, 64), kTr), (slice(64, 128), qTr)),
    }
    for tb in range(T // 512):
        sl = slice(tb * 512, (tb + 1) * 512)
        if tb == 0:
            # staged boot chunks: the first (wq_k0 + x_k0) unblocks the
            # very first matmul
            nc.sync.dma_start(boot_sb[:, 0:640], boot[:, 0:640])
            nc.sync.dma_start(boot_sb[:, 640:2048], boot[:, 640:2048])
            xtb = xbpool.tile([128, NK, 512], F16, tag="xtb")
            nc.sync.dma_start(xtb[:, 2:4, :], xT3[:, 2:4, sl])
            nc.sync.dma_start(xtb[:, 4:NK, :], xT3[:, 4:NK, sl])
            nc.sync.dma_start(wqkv_sb[:, :, 0:128], wqkv3[:, :, 0:128])
            nc.sync.dma_start(wqkv_sb[:, :, 128:512], wqkv3[:, :, 128:512])
            nc.sync.dma_start(m_sb[:],
                              masks.rearrange("p (a q) -> p a q", a=2))
            nc.sync.dma_start(wproj_sb[:],
                              wproj.rearrange("(k p) m -> p k m", p=128))
            xts_all[0] = boot_x + [xtb[:, k, :] for k in range(2, NK)]
            chain_qk(0, wq_ks, qk_dsts["wq"])
            chain_qk(0, wk_sb, qk_dsts["wk"])
            chain_qk(0, wqkr_sb, qk_dsts["wqkr"])
            for tt in range(4):
                chain_v(0, tt)
        if tb + 1 < 4:  # prefetch next x block
            nsl = slice((tb + 1) * 512, (tb + 2) * 512)
            nxtb = xbpool.tile([128, NK, 512], F16, tag="xtb")
            nc.sync.dma_start(nxtb[:], xT3[:, :, nsl])
            xts_all[tb + 1] = [nxtb[:, k, :] for k in range(NK)]
        full_attn(tb, micro=(chain(
            micro_chain_qk(tb + 1, wq_ks, qk_dsts["wq"]),
            micro_chain_qk(tb + 1, wk_sb, qk_dsts["wk"]))
            if tb + 1 < 4 else None))
        if tb == 3:
            cproj_tile(4)  # 2 dep-free tiles fill the full-norm window;
            cproj_tile(5)  # more would queue DVE copies ahead of masks
        micro = micro_chain_qk(tb + 1, wqkr_sb, qk_dsts["wqkr"]) \
            if tb + 1 < 4 else None
        red_pair(tb, micro=micro)
        if tb == 1:  # dep-free cproj covers the red normalize window
            for tt in range(0, 4):
                cproj_tile(tt)
        elif tb == 3:
            for tt in range(6, 12):
                cproj_tile(tt)
        if tb + 1 < 4:
            for tt in range(4):
                chain_v(tb + 1, tt)
    for tt in range(12, 16):
        cproj_tile(tt)
    if dbg is not None:
        for name, tile in (("dqTf", qTf), ("dkTf", kTf), ("dqTr", qTr),
                           ("dkTr", kTr), ("dyTf", yTf), ("dyTr", yTr)):
            nc.sync.dma_start(dbg[name], tile[:])
        nc.sync.dma_start(dbg["dv"], v_sb[:].rearrange("p a h q -> p (a h q)"))


def _build_nc(reps=1, debug_outs=False):
    nc = bacc.Bacc(trn_type="TRN2", target_bir_lowering=False, debug=False,
                   num_devices=1)

    xT = nc.dram_tensor("xT", [C, T], F16, kind="ExternalInput").ap()
    wqkv = nc.dram_tensor("wqkv", [C, 512], F16, kind="ExternalInput").ap()
    wproj = nc.dram_tensor("wproj", [256, C], F16, kind="ExternalInput").ap()
    masks = nc.dram_tensor("masks", [128, 256], F16,
                           kind="ExternalInput").ap()
    boot = nc.dram_tensor("boot", [128, 2048], F16,
                          kind="ExternalInput").ap()
    out = nc.dram_tensor("o", [T, C], F16, kind="ExternalOutput").ap()
    aps = (xT, wqkv, wproj, masks, boot, out)
    dbg = None
    if debug_outs:
        dbg = {}
        for name, shape in (("dqTf", [128, T]), ("dkTf", [128, T]),
                            ("dqTr", [64, T]), ("dkTr", [64, T]),
                            ("dyTf", [128, T]), ("dyTr", [128, T]),
                            ("dv", [128, T * 4])):
            dbg[name] = nc.dram_tensor(name, shape, F16,
                                       kind="ExternalOutput").ap()

    with TileContext(nc) as tc:
        with (
            tc.tile_pool(name="wpool", bufs=1) as wpool,
            tc.tile_pool(name="xbpool", bufs=2) as xbpool,
            tc.tile_pool(name="qk", bufs=1) as qkpool,
            tc.tile_pool(name="pf", bufs=2) as pfpool,
            tc.tile_pool(name="pr", bufs=2) as prpool,
            tc.tile_pool(name="opool", bufs=4) as opool,
            tc.tile_pool(name="rpool", bufs=2) as rpool,
            tc.tile_pool(name="ps_m", bufs=2, space="PSUM") as ps_m,
            tc.tile_pool(name="ps_s", bufs=2, space="PSUM") as ps_s,
            tc.tile_pool(name="ps_y", bufs=1, space="PSUM") as ps_y,
        ):
            pools = (wpool, xbpool, qkpool, pfpool, prpool, opool, rpool,
                     ps_m, ps_s, ps_y)
            for _ in range(reps):
                _emit_body(nc, pools, aps, dbg=dbg)

    nc.compile()
    return nc


_NC_CACHE = {}


def _get_nc(reps=1):
    if reps not in _NC_CACHE:
        _NC_CACHE[reps] = _build_nc(reps)
    return _NC_CACHE[reps]


_MASKS = None


def make_in_maps(x, w_qkv_full, w_qk_red, w_v_red, w_proj):
    global _MASKS
    if _MASKS is None:
        _MASKS = np.ascontiguousarray(host_masks().reshape(128, 256))
    x = np.asarray(x, np.float32)
    w_qkv_full = np.asarray(w_qkv_full, np.float32)
    w_qk_red = np.asarray(w_qk_red, np.float32)
    w_v_red = np.asarray(w_v_red, np.float32)
    w_proj = np.asarray(w_proj, np.float32)
    sf = np.float32(1.0 / np.sqrt(HDIM))
    sr = np.float32(1.0 / np.sqrt(RDIM))
    in_maps = []
    for c in range(N_CORES):
        b, g = divmod(c, 4)
        hA, hB = 2 * g, 2 * g + 1
        wq = np.concatenate([w_qkv_full[:, 64 * hA:64 * hA + 64],
                             w_qkv_full[:, 64 * hB:64 * hB + 64]], 1) * sf
        wk = np.concatenate([w_qkv_full[:, 512 + 64 * hA:512 + 64 * hA + 64],
                             w_qkv_full[:, 512 + 64 * hB:512 + 64 * hB + 64]], 1)
        # packed reduced projection: rows 0:32 krA | 32:64 krB | qrA | qrB
        wqkr = np.concatenate(
            [w_qk_red[:, 256 + 32 * hA:256 + 32 * hA + 32],
             w_qk_red[:, 256 + 32 * hB:256 + 32 * hB + 32],
             w_qk_red[:, 32 * hA:32 * hA + 32] * sr,
             w_qk_red[:, 32 * hB:32 * hB + 32] * sr], 1)
        wv = np.concatenate([w_qkv_full[:, 1024 + 64 * hA:1024 + 64 * hA + 64],
                             w_qkv_full[:, 1024 + 64 * hB:1024 + 64 * hB + 64],
                             w_v_red[:, 64 * hA:64 * hA + 64],
                             w_v_red[:, 64 * hB:64 * hB + 64]], 1)
        wp = np.concatenate([w_proj[64 * hA:64 * hA + 64, :],
                             w_proj[64 * hB:64 * hB + 64, :],
                             w_proj[512 + 64 * hA:512 + 64 * hA + 64, :],
                             w_proj[512 + 64 * hB:512 + 64 * hB + 64, :]], 0)
        wqkv = np.concatenate([wk, wqkr, wv], 1)
        xT16 = np.ascontiguousarray(x[b].T).astype(np.float16)
        wq16 = wq.astype(np.float16)
        # boot, interleaved: [wq_k0 | x_k0 | wq_k1 | x_k1 | wq_k2..7]
        wqk = wq16.reshape(NK, 128, 128)
        boot = np.concatenate(
            [wqk[0], xT16[0:128, 0:512], wqk[1], xT16[128:256, 0:512],
             wqk[2:].transpose(1, 0, 2).reshape(128, 768)], 1)
        in_maps.append({
            "xT": xT16,
            "wqkv": np.ascontiguousarray(wqkv).astype(np.float16),
            "wproj": np.ascontiguousarray(wp).astype(np.float16),
            "masks": _MASKS,
            "boot": np.ascontiguousarray(boot),
        })
    return in_maps


def kernel(x, w_qkv_full, w_qk_red, w_v_red, w_proj):
    nc = _get_nc()
    in_maps = make_in_maps(x, w_qkv_full, w_qk_red, w_v_red, w_proj)
    r = bass_utils.run_bass_kernel_spmd(nc, in_maps,
                                        core_ids=list(range(N_CORES)),
                                        trace=False)
    outs = [r.results[c]["o"] for c in range(N_CORES)]
    y = np.zeros((B, T, C), np.float32)
    for b in range(B):
        for j in range(4):
            y[b] += np.asarray(outs[4 * b + j], np.float32)
    return y
